# revision 23
# baseline (speedup 1.0000x reference)
"""Block-sparse linear y = x @ W^T on 8 Trainium2 NeuronCores.

Strategy: the 32x32 block structure (50% block density, random scatter) is not
exploitable on a 128x128 PE array (any packing at 32-granularity wastes more
PE volume than the ~39% merged density saves), so we densify W^T on the host
(cheap: 8MB of scatter-adds) and run a dense GEMM, sharded 4-way over tokens
x 2-way over out_features (8 cores, no collectives).

The GEMM runs as a 3-term fp8(e4m3) split in the PE's DoubleRow perf mode:
  y = x_hi@W_hi + x_hi@W_lo + x_lo@W_hi   (hi = fp8(v), lo = fp8(v - hi))
DoubleRow packs two K-values per partition (the moving bus carries 4 fp8/cycle
vs 1 bf16-pair... 2 elems), so each matmul contracts 256 K in 0.5 cycles/row:
a full K-pass is 13.7us/core and the 3-term split costs 41us — vs the 54.6us
single-pass floor of bf16/fp32r. Dropped term x_lo@W_lo plus residual
requantization leave ~1e-3 max-rel error vs the fp32 reference (tol 2e-2);
measured on this data: 1.04e-3 (bf16 single-pass is 2.0e-3).

Schedule per core: two out-column groups g of 512 features; per group the 8
PSUM banks hold y^T fragments [4 og x 128 outs, 2 th x 512 tokens] and
accumulate all 3 terms x 8 K-double-tiles (24 matmuls/bank, one start/stop
group). Loop order kd-outer/term-mid/og/th-inner keeps DMA demand flat at
~300GB/s (no front-loaded pass) and reuses each 256-wide fp8 stationary for
2 matmuls so LDWEIGHTS hides under the 214ns pair window. Each kd-step's
operands ([x_hi|x_lo|W_hi_g|W_lo_g] slabs) are host-packed into ONE bundle =
one linear DMA = one completion semaphore, so a k-step becomes ready
atomically (per-DMA completion order jitters +-1.5us, so splitting a step
across DMAs stalls the PE mid-step and drops the HAM clock). Group B's W
bundles prefetch behind the group-A stream; x slabs are reused from SBUF.
PSUM drains (vector copy + scalar-queue store of y^T, transposed back on the
host) chase each group's tail; the final bank splits its store across both
HWDGE queues to shorten the drain tail. Short warmup matmuls (tiny memset
dependency) keep the PE busy from preamble-end until the first bundle lands,
holding the HAM clock gate open so real matmuls start at full clock.
"""

import numpy as np

TOKENS, IN_F, OUT_F = 4096, 2048, 2048
BLOCK = 32
N_CORES = 8
TG, OG = 4, 2  # token groups x out-feature groups
T_SH = TOKENS // TG  # 1024 tokens per core
O_SH = OUT_F // OG  # 1024 out features per core
P = 128
NFREE = 512  # PSUM bank free dim (fp32)
KD = IN_F // (2 * P)  # 8 K-double-tiles of 256
OGL = 4  # out-feature 128-groups per column group
TH = 2  # token halves
GD = 2  # out-column groups of 512
TERMS = ((0, 0), (0, 1), (1, 0))  # (x variant, W variant): hh, hl, lh
N_WARM = 24  # PE clock-gate warmup matmuls, sized to the DMA head wait

TRACE = False  # set by test.py to capture an NTFF profile

_nc_cache = {}
_last_result = None  # BassKernelResults of the most recent run (for test.py)


def _build_nc():
    import concourse.mybir as mybir
    import concourse.tile as tile
    from concourse import bacc

    if "fp8" in _nc_cache:
        return _nc_cache["fp8"]

    f8 = mybir.dt.float8e4
    f32 = mybir.dt.float32
    DR = mybir.MatmulPerfMode.DoubleRow

    nc = bacc.Bacc(None, target_bir_lowering=False)
    # Host-pre-blocked inputs (exact SBUF layouts; all DMAs are linear):
    # bnA: per-kd main bundles [KD][P][4][2][512]
    #      r=0,1: x_hi token-halves, r=2: W_hi group-A cols, r=3: W_lo;
    #      [i] = K-pair slot (global k = kd*256 + i*128 + p)
    # bnX: per-kd x_lo bundles [KD][P][2][2][512] (needed 2 terms later, so
    #      they ride a second DMA whose deadline is 1.7us after bnA's)
    # bnB: group-B W bundles [KD][P][2][2][512] (v = hi/lo)
    bnA = nc.dram_tensor("bnA", [KD, P, 4, 2, NFREE], f8, kind="ExternalInput")
    bnX = nc.dram_tensor("bnX", [KD, P, 2, 2, NFREE], f8, kind="ExternalInput")
    bnB = nc.dram_tensor("bnB", [KD, P, 2, 2, NFREE], f8, kind="ExternalInput")
    yT = nc.dram_tensor("yT", [O_SH, T_SH], f32, kind="ExternalOutput")

    with tile.TileContext(nc) as tc:
        with (
            tc.tile_pool(name="xp", bufs=1) as xp,
            tc.tile_pool(name="wp", bufs=1) as wp,
            tc.tile_pool(name="op", bufs=1) as op,
            tc.tile_pool(name="ps", bufs=1, space="PSUM") as ps,
        ):
            # Warm the PE's HAM clock gate during the initial DMA head wait.
            zt = xp.tile([P, 2, NFREE], f8, tag="warm", name="warm")
            nc.gpsimd.memset(zt[:], 0.0)
            warm_ps = ps.tile([P, NFREE], f32, tag="ps0", name="warm_ps")
            for _ in range(N_WARM):
                nc.tensor.matmul(
                    warm_ps[:], zt[:, :, :P], zt[:], start=True, stop=True,
                    perf_mode=DR,
                )

            def psums():
                return [
                    ps.tile([P, NFREE], f32, tag=f"ps{b}", name=f"ps{b}")
                    for b in range(OGL * TH)
                ]

            tA = [None] * KD  # [x_hi | W_hi_A | W_lo_A] bundle tiles
            tX = [None] * KD  # x_lo bundle tiles

            def xsrc(kd, xv, th):
                t = tA[kd] if xv == 0 else tX[kd]
                return t[:, th, :, :]

            def emit_group(g, wtile):
                """All 24 matmuls per bank for out-column group g.

                wtile(kd, v) -> stationary source AP [P, 2, NFREE] holding
                W variant v's columns for this group.
                """
                grp = psums()
                for kd in range(KD):
                    if g == 0:
                        t = xp.tile(
                            [P, 4, 2, NFREE], f8, tag=f"bnA{kd}", name=f"bnA{kd}"
                        )
                        nc.sync.dma_start(t[:], bnA[kd])
                        tA[kd] = t
                        t = xp.tile(
                            [P, 2, 2, NFREE], f8, tag=f"bnX{kd}", name=f"bnX{kd}"
                        )
                        nc.sync.dma_start(t[:], bnX[kd])
                        tX[kd] = t
                    for ti, (xv, wv) in enumerate(TERMS):
                        for og in range(OGL):
                            lw = wtile(kd, wv)[:, :, og * P : (og + 1) * P]
                            for th in range(TH):
                                nc.tensor.matmul(
                                    grp[og * TH + th][:],
                                    lw,
                                    xsrc(kd, xv, th),
                                    start=(kd == 0 and ti == 0),
                                    stop=(kd == KD - 1 and ti == len(TERMS) - 1),
                                    perf_mode=DR,
                                )
                return grp

            def drain(grp, g, last_split):
                """Evict a group's psums: copies alternate DVE/Act engines,
                stores alternate the scalar/sync HWDGE queues, so the 3.4us
                of drain work clears the banks ~2x faster than one engine."""
                for b in range(OGL * TH):
                    og, th = divmod(b, TH)
                    rows = slice(g * NFREE + og * P, g * NFREE + (og + 1) * P)
                    ot = op.tile([P, NFREE], f32, tag=f"o{g}_{b}", name=f"o{g}_{b}")
                    cp = nc.vector.tensor_copy if b % 2 == 0 else nc.scalar.copy
                    dq = nc.sync if b % 2 == 0 else nc.scalar
                    if last_split and b == OGL * TH - 1:
                        # last bank: halves pipeline across both engines/queues
                        for h in range(2):
                            sl = slice(h * (NFREE // 2), (h + 1) * (NFREE // 2))
                            (nc.vector.tensor_copy if h == 0 else nc.scalar.copy)(
                                ot[:, sl], grp[b][:, sl]
                            )
                            (nc.scalar if h == 0 else nc.sync).dma_start(
                                yT[
                                    rows,
                                    th * NFREE + h * (NFREE // 2) : th * NFREE
                                    + (h + 1) * (NFREE // 2),
                                ],
                                ot[:, sl],
                            )
                    else:
                        cp(ot[:], grp[b][:])
                        dq.dma_start(
                            yT[rows, th * NFREE : (th + 1) * NFREE], ot[:]
                        )

            # ---- Group A: out cols 0-511; bundles stream JIT ----
            grpA = emit_group(0, lambda kd, v: tA[kd][:, 2 + v, :, :])

            # Group B W bundles prefetch behind the group-A stream
            tB = []
            for kd in range(KD):
                t = wp.tile([P, 2, 2, NFREE], f8, tag=f"bnB{kd}", name=f"bnB{kd}")
                nc.sync.dma_start(t[:], bnB[kd])
                tB.append(t)

            drain(grpA, 0, last_split=False)

            # ---- Group B: out cols 512-1023; x reused from SBUF ----
            grpB = emit_group(1, lambda kd, v: tB[kd][:, v, :, :])
            drain(grpB, 1, last_split=True)

    nc.compile()
    _nc_cache["fp8"] = nc
    return nc


def _densify_wT(weight_blocks, block_rows, block_cols):
    """Scatter-add the 32x32 blocks into dense W^T [in_features, out_features]."""
    nc_blk = IN_F // BLOCK
    nr_blk = OUT_F // BLOCK
    wcr = np.zeros((nc_blk, nr_blk, BLOCK, BLOCK), np.float32)
    # block b occupies W[32r:32r+32, 32c:32c+32]; W^T gets the transposed block
    np.add.at(
        wcr,
        (block_cols.astype(np.int64), block_rows.astype(np.int64)),
        np.swapaxes(weight_blocks.astype(np.float32, copy=False), 1, 2),
    )
    return np.ascontiguousarray(wcr.transpose(0, 2, 1, 3).reshape(IN_F, OUT_F))


def _pack_core_inputs(xT_sh, wT_sh):
    """Pack one core's x^T and W^T shards into fp8 hi/lo DMA bundles.

    K mapping: global k = kd*256 + i*128 + p for both operands.
    """
    import ml_dtypes

    f8 = np.dtype(ml_dtypes.float8_e4m3)
    X = np.ascontiguousarray(xT_sh).reshape(KD, 2, P, T_SH)  # [kd, i, p, t]
    W = np.ascontiguousarray(wT_sh).reshape(KD, 2, P, O_SH)  # [kd, i, p, o]
    xh = X.astype(f8)
    xl = (X - xh.astype(np.float32)).astype(f8)
    wh = W.astype(f8)
    wl = (W - wh.astype(np.float32)).astype(f8)

    def xslab(a, th):  # [kd, p, i, 512]
        return a[:, :, :, th * NFREE : (th + 1) * NFREE].transpose(0, 2, 1, 3)

    def wslab(a, g):
        return a[:, :, :, g * NFREE : (g + 1) * NFREE].transpose(0, 2, 1, 3)

    def bundle(slabs):  # [KD, R, P, 2, 512] -> [KD, P, R, 2, 512]
        return np.ascontiguousarray(
            np.stack(slabs, axis=1).transpose(0, 2, 1, 3, 4)
        )

    return {
        "bnA": bundle([xslab(xh, 0), xslab(xh, 1), wslab(wh, 0), wslab(wl, 0)]),
        "bnX": bundle([xslab(xl, 0), xslab(xl, 1)]),
        "bnB": bundle([wslab(wh, 1), wslab(wl, 1)]),
    }


def kernel(x, weight_blocks, block_rows, block_cols):
    global _last_result
    from concourse.bass_utils import run_bass_kernel_spmd

    x = np.asarray(x, dtype=np.float32)
    wT = _densify_wT(
        np.asarray(weight_blocks), np.asarray(block_rows), np.asarray(block_cols)
    )
    xT = np.ascontiguousarray(x.T)

    in_maps = []
    for c in range(N_CORES):
        tg, og = divmod(c, OG)
        in_maps.append(
            _pack_core_inputs(
                xT[:, tg * T_SH : (tg + 1) * T_SH],
                wT[:, og * O_SH : (og + 1) * O_SH],
            )
        )

    nc = _build_nc()
    res = None
    for attempt in range(3):  # transient NRT device errors happen; retry
        try:
            res = run_bass_kernel_spmd(
                nc, in_maps, core_ids=list(range(N_CORES)), trace=TRACE
            )
            break
        except Exception:
            if attempt == 2:
                raise
            import time

            time.sleep(3)
    _last_result = res

    y = np.empty((TOKENS, OUT_F), np.float32)
    for c in range(N_CORES):
        tg, og = divmod(c, OG)
        y[tg * T_SH : (tg + 1) * T_SH, og * O_SH : (og + 1) * O_SH] = (
            res.results[c]["yT"].T
        )
    return y


# revision 24
# speedup vs baseline: 1.4514x; 1.4514x over previous
"""Block-sparse linear y = x @ W^T on 8 Trainium2 NeuronCores.

Strategy: the 32x32 block structure (50% block density, random scatter) is not
exploitable on a 128x128 PE array (any packing at 32-granularity wastes more
PE volume than the ~39% merged density saves), so we densify W^T on the host
(cheap: 8MB of scatter-adds) and run a dense GEMM, sharded 4-way over tokens
x 2-way over out_features (8 cores, no collectives).

The matmuls run in bfloat16 (PE rate is identical to float32r at 1.0
cycles/row, so the PE floor is 131072 cycles = 54.6us/core either way), which
halves input HBM traffic to 8MB/core and takes the DMA stream off the PE's
critical path — the fp32r version sat exactly at the 20MB/55us ridge and any
DMA jitter stalled the PE. bf16 rounding error is ~2e-3 max-rel vs the fp32
reference (tolerance 2e-2).

Schedule per core, two passes over the out-feature halves:
(1) n=0, k-outer/m-inner. Each k-step's x tile and W tile are host-packed
    into ONE bundle = one linear DMA = one completion semaphore, so a k-step
    becomes ready atomically (per-DMA completion order jitters +-1.5us, so
    splitting a k-step across DMAs stalls the PE mid-step and drops the HAM
    clock).
(2) n=1, m-outer/k-inner: everything is SBUF-resident by now (the n=1 W half
    prefetches as two 1MB DMAs behind the n=0 stream), so each bank runs its
    16 matmuls back-to-back and drains (vector copy + store) while the next
    bank computes. The last bank's store splits across the two HWDGE queues
    to halve the drain tail.
All input DMAs ride the sync-engine queue in consumption order; y stores ride
the scalar-engine queue so stores never delay loads. Short warmup matmuls
(tiny memset dependency) keep the PE busy from preamble-end until the first
bundle lands, holding the HAM clock gate open so real matmuls start at full
clock.
"""

import numpy as np

TOKENS, IN_F, OUT_F = 4096, 2048, 2048
BLOCK = 32
N_CORES = 8
TG, OG = 4, 2  # token groups x out-feature groups
T_SH = TOKENS // TG  # 1024 tokens per core
O_SH = OUT_F // OG  # 1024 out features per core
P = 128
NFREE = 512  # PSUM bank free dim (fp32)
KT = IN_F // P  # 16 k tiles
MT = T_SH // P  # 8 psum banks
XH = T_SH // 2  # token half (k=0/k=1 head bundles)
N_WARM = 22  # PE clock-gate warmup matmuls (~150ns each, sized to DMA head)

MM_DTYPE = "bfloat16"  # "bfloat16" (fast DMA) or "float32r" (exact-ish)
TRACE = False  # set by test.py to capture an NTFF profile

_nc_cache = {}
_last_result = None  # BassKernelResults of the most recent run (for test.py)


def _build_nc():
    import concourse.mybir as mybir
    import concourse.tile as tile
    from concourse import bacc

    key = MM_DTYPE
    if key in _nc_cache:
        return _nc_cache[key]

    dt_mm = getattr(mybir.dt, MM_DTYPE)
    f32 = mybir.dt.float32

    nc = bacc.Bacc(None, target_bir_lowering=False)
    # Host-pre-blocked inputs (exact SBUF layouts; all DMAs are linear):
    # bn: per-k bundles [KT][P][T_SH + NFREE] = [x^T k-tile | w n0 k-tile]
    # w1: n=1 W^T supertile [P][KT][NFREE]
    bn = nc.dram_tensor("bn", [KT, P, T_SH + NFREE], dt_mm, kind="ExternalInput")
    w1q = nc.dram_tensor("w1q", [P, KT, NFREE], dt_mm, kind="ExternalInput")
    y = nc.dram_tensor("y", [T_SH, O_SH], f32, kind="ExternalOutput")

    with tile.TileContext(nc) as tc:
        with (
            tc.tile_pool(name="xp", bufs=1) as xp,
            tc.tile_pool(name="wp", bufs=1) as wp,
            tc.tile_pool(name="op", bufs=1) as op,
            tc.tile_pool(name="ps", bufs=1, space="PSUM") as ps,
        ):
            # Warm the PE's HAM clock gate during the initial DMA head wait.
            zt = xp.tile([P, P], dt_mm, tag="warm", name="warm")
            nc.gpsimd.memset(zt[:], 0.0)
            warm_ps = ps.tile([P, NFREE], f32, tag="ps0", name="warm_ps")
            for _ in range(N_WARM):
                nc.tensor.matmul(warm_ps[:, :P], zt[:], zt[:], start=True, stop=True)

            bnt = [None] * KT  # bundle tiles [P, T_SH + NFREE]

            def lhsT(m, k):
                """Stationary x^T slice for bank m, k-tile k."""
                return bnt[k][:, m * P : (m + 1) * P]

            def psums():
                return [
                    ps.tile([P, NFREE], f32, tag=f"ps{m}", name=f"ps{m}")
                    for m in range(MT)
                ]

            # ---- Pass 1: n=0, k-outer/m-inner, bundles streamed JIT ----
            ps0 = psums()
            for k in range(KT):
                t = xp.tile([P, T_SH + NFREE], dt_mm, tag=f"bn{k}", name=f"bn{k}")
                nc.sync.dma_start(t[:], bn[k])
                bnt[k] = t
                for m in range(MT):
                    nc.tensor.matmul(
                        ps0[m][:],
                        lhsT(m, k),
                        t[:, T_SH : T_SH + NFREE],
                        start=(k == 0),
                        stop=(k == KT - 1),
                    )

            # n=1 W half: two 1MB prefetches queued behind the n=0 stream
            w1 = []
            for h in range(2):
                wt = wp.tile([P, KT // 2, NFREE], dt_mm, tag=f"w1_{h}", name=f"w1_{h}")
                nc.sync.dma_start(
                    wt[:], w1q[:, h * (KT // 2) : (h + 1) * (KT // 2), :]
                )
                w1.append(wt)

            for m in range(MT):  # evict n=0 psums; y stores on the scalar queue
                ot = op.tile([P, NFREE], f32, tag=f"o0_{m}", name=f"o0_{m}")
                nc.vector.tensor_copy(ot[:], ps0[m][:])
                nc.scalar.dma_start(y[m * P : (m + 1) * P, 0:NFREE], ot[:])

            # ---- Pass 2: n=1, m-outer/k-inner; each bank drains as it ends ----
            ps1 = psums()
            for m in range(MT):
                for k in range(KT):
                    nc.tensor.matmul(
                        ps1[m][:],
                        lhsT(m, k),
                        w1[k // (KT // 2)][:, k % (KT // 2), :],
                        start=(k == 0),
                        stop=(k == KT - 1),
                    )
                ot = op.tile([P, NFREE], f32, tag=f"o1_{m}", name=f"o1_{m}")
                if m == MT - 1:
                    # last bank: drain in halves on BOTH hwdge queues so the
                    # copies, store issues, and wire time all pipeline
                    for h in range(2):
                        sl = slice(h * (NFREE // 2), (h + 1) * (NFREE // 2))
                        nc.vector.tensor_copy(ot[:, sl], ps1[m][:, sl])
                        eng = nc.scalar if h == 0 else nc.sync
                        eng.dma_start(
                            y[
                                m * P : (m + 1) * P,
                                NFREE + h * (NFREE // 2) : NFREE
                                + (h + 1) * (NFREE // 2),
                            ],
                            ot[:, sl],
                        )
                else:
                    nc.vector.tensor_copy(ot[:], ps1[m][:])
                    nc.scalar.dma_start(
                        y[m * P : (m + 1) * P, NFREE : 2 * NFREE], ot[:]
                    )

    nc.compile()
    _nc_cache[key] = nc
    return nc


def _densify_wT(weight_blocks, block_rows, block_cols):
    """Scatter-add the 32x32 blocks into dense W^T [in_features, out_features]."""
    nc_blk = IN_F // BLOCK
    nr_blk = OUT_F // BLOCK
    wcr = np.zeros((nc_blk, nr_blk, BLOCK, BLOCK), np.float32)
    # block b occupies W[32r:32r+32, 32c:32c+32]; W^T gets the transposed block
    np.add.at(
        wcr,
        (block_cols.astype(np.int64), block_rows.astype(np.int64)),
        np.swapaxes(weight_blocks.astype(np.float32, copy=False), 1, 2),
    )
    return np.ascontiguousarray(wcr.transpose(0, 2, 1, 3).reshape(IN_F, OUT_F))


def _mm_np_dtype():
    if MM_DTYPE == "bfloat16":
        import ml_dtypes

        return np.dtype(ml_dtypes.bfloat16)
    return np.dtype(np.float32)


def _pack_core_inputs(xT_sh, wT_sh):
    """Block one core's x^T and W^T shards into the kernel's DMA layouts."""
    dt = _mm_np_dtype()
    X = xT_sh.reshape(KT, P, T_SH).astype(dt)  # [k, p, t]
    W = wT_sh.reshape(KT, P, 2, NFREE).transpose(2, 0, 1, 3).astype(dt)  # [n,k,p,o]
    bn = np.concatenate([X, W[0]], axis=2)  # [KT, P, T_SH + NFREE]
    w1 = np.ascontiguousarray(W[1].transpose(1, 0, 2))  # [P, KT, NFREE]
    return {"bn": np.ascontiguousarray(bn), "w1q": w1}


def kernel(x, weight_blocks, block_rows, block_cols):
    global _last_result
    from concourse.bass_utils import run_bass_kernel_spmd

    x = np.asarray(x, dtype=np.float32)
    wT = _densify_wT(
        np.asarray(weight_blocks), np.asarray(block_rows), np.asarray(block_cols)
    )
    xT = np.ascontiguousarray(x.T)

    in_maps = []
    for c in range(N_CORES):
        tg, og = divmod(c, OG)
        in_maps.append(
            _pack_core_inputs(
                xT[:, tg * T_SH : (tg + 1) * T_SH],
                wT[:, og * O_SH : (og + 1) * O_SH],
            )
        )

    nc = _build_nc()
    res = None
    for attempt in range(3):  # transient NRT device errors happen; retry
        try:
            res = run_bass_kernel_spmd(
                nc, in_maps, core_ids=list(range(N_CORES)), trace=TRACE
            )
            break
        except Exception:
            if attempt == 2:
                raise
            import time

            time.sleep(3)
    _last_result = res

    y = np.empty((TOKENS, OUT_F), np.float32)
    for c in range(N_CORES):
        tg, og = divmod(c, OG)
        y[tg * T_SH : (tg + 1) * T_SH, og * O_SH : (og + 1) * O_SH] = res.results[c][
            "y"
        ]
    return y
